# revision 1
# baseline (speedup 1.0000x reference)
"""MLA (multi-head latent attention) Trainium2 Bass kernel, 8-core SPMD.

Sharding: 2-way data parallel over batch x 4-way tensor parallel over heads.
Core c handles batch b = c // 4 and heads [hg*4, hg*4+4) with hg = c % 4.
Each core computes the full MLA forward for its batch/heads and the partial
out-projection (row-sharded W_o); the host sums the 4 partials per batch.

Layouts on device (partition dim first):
  xT      [128, 16, S]   x[b].T, feature-on-partition
  c_qT    [128, 4, S]    (x @ W_dq).T
  c_kvT   [128, 4, S]
  krx1/2  [128, S]       (x @ W_kr)_cols.T, rope halves split, 4 heads x 32
  qT/kT   [128, 4, S]    per head: rows 0:64 content, 64:128 rope
  v       [128, 16, 512] [token%128, token//128, head*128+d]
  scoresT [128k, 512q]   PSUM, exp'd -> expT; Z via ones-matmul; normalize at
                         attn-out eviction with a broadcasted 1/Z.
All matmul operands are float32r (full PE rate at N>=256, ~1.5e-4 rel err).
"""
import sys

sys.path.insert(0, "/opt/trn_rl_repo")

import numpy as np

import concourse.bacc as bacc
import concourse.mybir as mybir
import concourse.tile as tile
from concourse import bass_utils

H_DIM = 2048
N_HEADS = 16
D_HEAD = 128
D_ROPE = 64
D_NOPE = 64
HALF = D_ROPE // 2          # 32
C_DIM = 512
ROPE_BASE = 10000.0
HPC = 4                     # heads per core
B = 2
S_FULL = 2048
KC = H_DIM // 128           # 16
CC = C_DIM // 128           # 4
SCALE = 1.0 / float(np.sqrt(D_HEAD))
NEG = -1e9

f32r = mybir.dt.float16   # matmul operand dtype (1 cyc/row, FWL)
f32 = mybir.dt.float32
bf16 = mybir.dt.float16


def build_nc(S=S_FULL):
    TC = S // 512            # stage-1 token chunks
    QG = S // 512            # query groups
    TT = S // 128            # token tiles

    nc = bacc.Bacc("TRN2", target_bir_lowering=False, debug=False)

    d_xT = nc.dram_tensor("xT", [TC, 128, KC * 512], f32r, kind="ExternalInput")
    d_wdq = nc.dram_tensor("wdq", [128, KC, C_DIM], f32r, kind="ExternalInput")
    d_wdkv = nc.dram_tensor("wdkv", [128, KC, C_DIM], f32r, kind="ExternalInput")
    d_wkrx1 = nc.dram_tensor("wkrx1", [128, KC, HPC * HALF], f32r, kind="ExternalInput")
    d_wkrx2 = nc.dram_tensor("wkrx2", [128, KC, HPC * HALF], f32r, kind="ExternalInput")
    d_wuq = nc.dram_tensor("wuq", [128, CC, HPC * D_NOPE], f32r, kind="ExternalInput")
    d_wuk = nc.dram_tensor("wuk", [128, CC, HPC * D_NOPE], f32r, kind="ExternalInput")
    d_wqrx1 = nc.dram_tensor("wqrx1", [128, CC, HPC * HALF], f32r, kind="ExternalInput")
    d_wqrx2 = nc.dram_tensor("wqrx2", [128, CC, HPC * HALF], f32r, kind="ExternalInput")
    d_wuv = nc.dram_tensor("wuv", [128, CC, HPC * D_HEAD], f32r, kind="ExternalInput")
    d_wo = nc.dram_tensor("wo", [128, HPC, H_DIM], f32r, kind="ExternalInput")
    d_cos = nc.dram_tensor("cosA", [128, S], bf16, kind="ExternalInput")
    d_sin = nc.dram_tensor("sinA", [128, S], bf16, kind="ExternalInput")
    d_mask = nc.dram_tensor("masks", [128, 4, 512], f32, kind="ExternalInput")
    d_onec = nc.dram_tensor("onec", [128, 1], f32r, kind="ExternalInput")
    d_oner = nc.dram_tensor("oner", [1, 128], f32r, kind="ExternalInput")
    d_o = nc.dram_tensor("o", [S, H_DIM], f32, kind="ExternalOutput")

    with tile.TileContext(nc) as tc:
        _cms = {}

        def popen(name, **kw):
            cm = tc.tile_pool(name=name, **kw)
            _cms[name] = cm
            return cm.__enter__()

        def pclose(*names):
            for n in names:
                _cms.pop(n).__exit__(None, None, None)

        # ---- persistent pools (manual lifetime management) ----
        p_const = popen("const", bufs=1)
        p_ckv = popen("ckv", bufs=1, side="right")
        p_krx = popen("krx", bufs=1, side="right")
        p_cq = popen("cq", bufs=1, side="right")

        onec = p_const.tile([128, 1], f32r, tag="onec")
        nc.sync.dma_start(onec[:], d_onec.ap())
        oner = p_const.tile([1, 128], f32r, tag="oner")
        nc.sync.dma_start(oner[:], d_oner.ap())

        cq = p_cq.tile([128, CC, S], f32r, tag="cq")
        ckv = p_ckv.tile([128, CC, S], f32r, tag="ckv")
        krx1 = p_krx.tile([128, S], bf16, tag="krx1")
        krx2 = p_krx.tile([128, S], bf16, tag="krx2")

        # ================= stage 1: compressions =================
        p_w1 = popen("w1", bufs=1, side="right")
        p_x = popen("xp", bufs=2, side="right")
        p_ps12 = popen("ps12", bufs=4, space="PSUM")

        wdq = p_w1.tile([128, KC, C_DIM], f32r, tag="wdq")
        nc.sync.dma_start(wdq[:], d_wdq.ap())
        wdkv = p_w1.tile([128, KC, C_DIM], f32r, tag="wdkv")
        nc.sync.dma_start(wdkv[:], d_wdkv.ap())
        wkrx1 = p_w1.tile([128, KC, HPC * HALF], f32r, tag="wkrx1")
        nc.sync.dma_start(wkrx1[:], d_wkrx1.ap())
        wkrx2 = p_w1.tile([128, KC, HPC * HALF], f32r, tag="wkrx2")
        nc.sync.dma_start(wkrx2[:], d_wkrx2.ap())

        for tci in range(TC):
            ts = slice(tci * 512, (tci + 1) * 512)
            xt = p_x.tile([128, KC, 512], f32r, tag="xt")
            nc.sync.dma_start(xt[:], d_xT.ap()[tci])
            for w_sb, out_sb in ((wdq, cq), (wdkv, ckv)):
                for m in range(CC):
                    ps = p_ps12.tile([128, 512], f32, tag="ps")
                    for k in range(KC):
                        nc.tensor.matmul(
                            ps[:],
                            w_sb[:, k, m * 128:(m + 1) * 128],
                            xt[:, k, :],
                            start=(k == 0), stop=(k == KC - 1),
                        )
                    nc.scalar.copy(out_sb[:, m, ts], ps[:])
            for w_sb, out_sb in ((wkrx1, krx1), (wkrx2, krx2)):
                ps = p_ps12.tile([128, 512], f32, tag="ps")
                for k in range(KC):
                    nc.tensor.matmul(
                        ps[:], w_sb[:, k, :], xt[:, k, :],
                        start=(k == 0), stop=(k == KC - 1),
                    )
                nc.scalar.copy(out_sb[:, ts], ps[:])

        pclose("xp", "w1")

        # ================= stage 2: up-projections + rope =================
        p_w2 = popen("w2", bufs=1, side="right")
        p_cs = popen("cs", bufs=1, side="right")
        p_tmp = popen("tmp", bufs=1, side="right")
        p_qT = popen("qT", bufs=1)

        wuq = p_w2.tile([128, CC, HPC * D_NOPE], f32r, tag="wuq")
        nc.sync.dma_start(wuq[:], d_wuq.ap())
        wuk = p_w2.tile([128, CC, HPC * D_NOPE], f32r, tag="wuk")
        nc.sync.dma_start(wuk[:], d_wuk.ap())
        wqrx1 = p_w2.tile([128, CC, HPC * HALF], f32r, tag="wqrx1")
        nc.sync.dma_start(wqrx1[:], d_wqrx1.ap())
        wqrx2 = p_w2.tile([128, CC, HPC * HALF], f32r, tag="wqrx2")
        nc.sync.dma_start(wqrx2[:], d_wqrx2.ap())
        wuv = p_w2.tile([128, CC, HPC * D_HEAD], f32r, tag="wuv")
        nc.sync.dma_start(wuv[:], d_wuv.ap())
        cosA = p_cs.tile([128, S], bf16, tag="cosA")
        nc.sync.dma_start(cosA[:], d_cos.ap())
        sinA = p_cs.tile([128, S], bf16, tag="sinA")
        nc.sync.dma_start(sinA[:], d_sin.ap())

        qT = p_qT.tile([128, HPC, S], f32r, tag="qT")

        def up_nope(w_sb, src, dst, g):
            gs = slice(g * 512, (g + 1) * 512)
            for m2 in range(2):
                ps = p_ps12.tile([128, 512], f32, tag="ps")
                for k in range(CC):
                    nc.tensor.matmul(
                        ps[:], w_sb[:, k, m2 * 128:(m2 + 1) * 128],
                        src[:, k, gs], start=(k == 0), stop=(k == CC - 1),
                    )
                nc.scalar.copy(dst[0:64, 2 * m2, gs], ps[0:64, :])
                nc.scalar.copy(dst[0:64, 2 * m2 + 1, gs], ps[64:128, :])

        def rope_from_psum(x1ap, x2ap, dst, g):
            # x1/x2: [128 = 4h*32, 512] (psum or sbuf); writes rope rows of dst
            gs = slice(g * 512, (g + 1) * 512)
            t1 = p_tmp.tile([128, 512], f32, tag="t1")
            t2 = p_tmp.tile([128, 512], f32, tag="t2")
            o1 = p_tmp.tile([128, 512], f32r, tag="o1")
            o2 = p_tmp.tile([128, 512], f32r, tag="o2")
            nc.vector.tensor_mul(t1[:], x1ap, cosA[:, gs])
            nc.vector.tensor_mul(t2[:], x2ap, sinA[:, gs])
            nc.vector.tensor_sub(o1[:], t1[:], t2[:])
            nc.vector.tensor_mul(t1[:], x1ap, sinA[:, gs])
            nc.vector.tensor_mul(t2[:], x2ap, cosA[:, gs])
            nc.vector.tensor_add(o2[:], t1[:], t2[:])
            for h in range(HPC):
                hs = slice(h * HALF, (h + 1) * HALF)
                nc.scalar.copy(dst[64:96, h, gs], o1[hs, :])
                nc.scalar.copy(dst[96:128, h, gs], o2[hs, :])

        # ---- queries (need cq) ----
        for g in range(QG):
            gs = slice(g * 512, (g + 1) * 512)
            up_nope(wuq, cq, qT, g)
            ps1 = p_ps12.tile([128, 512], f32, tag="ps")
            for k in range(CC):
                nc.tensor.matmul(ps1[:], wqrx1[:, k, :], cq[:, k, gs],
                                 start=(k == 0), stop=(k == CC - 1))
            ps2 = p_ps12.tile([128, 512], f32, tag="ps")
            for k in range(CC):
                nc.tensor.matmul(ps2[:], wqrx2[:, k, :], cq[:, k, gs],
                                 start=(k == 0), stop=(k == CC - 1))
            rope_from_psum(ps1[:], ps2[:], qT, g)


        # ---- keys + values (need ckv, krx) ----
        p_kT = popen("kT", bufs=1)
        p_v = popen("vp", bufs=1)
        kT = p_kT.tile([128, HPC, S], f32r, tag="kT")
        v_sb = p_v.tile([128, TT, 512], f32r, tag="v")

        for g in range(QG):
            gs = slice(g * 512, (g + 1) * 512)
            up_nope(wuk, ckv, kT, g)
            rope_from_psum(krx1[:, gs], krx2[:, gs], kT, g)
        for tt in range(TT):
            ps = p_ps12.tile([128, 512], f32, tag="ps")
            for k in range(CC):
                nc.tensor.matmul(ps[:], ckv[:, k, tt * 128:(tt + 1) * 128],
                                 wuv[:, k, :], start=(k == 0), stop=(k == CC - 1))
            nc.scalar.copy(v_sb[:, tt, :], ps[:])

        pclose("tmp", "cs", "w2", "cq", "krx", "ckv", "ps12")

        # ================= stage 3+4: attention + out-projection =================
        p_msk = popen("msk", bufs=1)
        p_wo = popen("wo", bufs=1)
        p_wk = popen("wk", bufs=8)
        p_att = popen("att", bufs=2)
        p_rz = popen("rz", bufs=2)
        p_o = popen("op", bufs=3)
        p_ps_s = popen("ps_s", bufs=4, space="PSUM")
        p_ps_a = popen("ps_a", bufs=2, space="PSUM")
        p_ps_z = popen("ps_z", bufs=1, space="PSUM")
        p_ps_b = popen("ps_b", bufs=1, space="PSUM")

        masks = p_msk.tile([128, 4, 512], f32, tag="masks")
        nc.sync.dma_start(masks[:], d_mask.ap())
        wo = p_wo.tile([128, HPC, H_DIM], f32r, tag="wo")
        nc.sync.dma_start(wo[:], d_wo.ap())

        for g in range(QG):
            gs = slice(g * 512, (g + 1) * 512)
            attn_g = p_att.tile([128, HPC, 512], f32r, tag="attn")
            for h in range(HPC):
                nkt = 4 * (g + 1)
                po = p_ps_a.tile([128, 512], f32, tag="po")
                za = p_wk.tile([128, 512], f32, tag="wk")
                for kt in range(nkt):
                    ps = p_ps_s.tile([128, 512], f32, tag="ps")
                    nc.tensor.matmul(ps[:], kT[:, h, kt * 128:(kt + 1) * 128],
                                     qT[:, h, gs], start=True, stop=True)
                    d = kt - 4 * g
                    if d >= 0:
                        nc.vector.tensor_add(ps[:], ps[:], masks[:, d, :])
                    et = p_wk.tile([128, 512], f32r, tag="wk")
                    nc.scalar.activation(et[:], ps[:],
                                         mybir.ActivationFunctionType.Exp,
                                         scale=SCALE)
                    if kt == 0:
                        nc.vector.tensor_copy(za[:], et[:])
                    else:
                        nc.vector.tensor_add(za[:], za[:], et[:])
                    nc.tensor.matmul(po[:], v_sb[:, kt, h * 128:(h + 1) * 128],
                                     et[:], start=(kt == 0), stop=(kt == nkt - 1))
                zr = p_wk.tile([128, 512], f32r, tag="wk")
                nc.vector.tensor_copy(zr[:], za[:])
                pz = p_ps_z.tile([1, 512], f32, tag="pz")
                nc.tensor.matmul(pz[:], onec[:], zr[:], start=True, stop=True)
                r0 = p_rz.tile([1, 512], f32, tag="r0")
                nc.vector.reciprocal(r0[:], pz[:])
                rzc = p_rz.tile([1, 512], f32r, tag="rzc")
                nc.vector.tensor_copy(rzc[:], r0[:])
                pb = p_ps_b.tile([128, 512], f32, tag="pb")
                nc.tensor.matmul(pb[:], oner[:], rzc[:], start=True, stop=True)
                rb = p_wk.tile([128, 512], f32, tag="wk")
                nc.scalar.copy(rb[:], pb[:])
                nc.vector.tensor_mul(attn_g[:, h, :], po[:], rb[:])
            # out-projection for this query group (4 token tiles)
            for t4 in range(4):
                tt = g * 4 + t4
                for nck in range(4):
                    pso = p_ps_s.tile([128, 512], f32, tag="ps")
                    for h in range(HPC):
                        nc.tensor.matmul(
                            pso[:], attn_g[:, h, t4 * 128:(t4 + 1) * 128],
                            wo[:, h, nck * 512:(nck + 1) * 512],
                            start=(h == 0), stop=(h == HPC - 1),
                        )
                    ot = p_o.tile([128, 512], f32, tag="ot")
                    nc.scalar.copy(ot[:], pso[:])
                    nc.sync.dma_start(
                        d_o.ap()[tt * 128:(tt + 1) * 128, nck * 512:(nck + 1) * 512],
                        ot[:])

        pclose("ps_b", "ps_z", "ps_a", "ps_s", "op", "rz", "att", "wk",
               "wo", "msk", "vp", "kT", "qT", "const")

    nc.compile()
    return nc


# ================= host-side prep =================

def _rope_tables(S):
    inv_freq = 1.0 / (ROPE_BASE ** (np.arange(HALF, dtype=np.float64) / HALF))
    ang = np.arange(S, dtype=np.float64)[:, None] * inv_freq[None, :]   # [S, 32]
    cosA = np.tile(np.cos(ang).T, (4, 1)).astype(np.float16)           # [128, S]
    sinA = np.tile(np.sin(ang).T, (4, 1)).astype(np.float16)
    return cosA, sinA


def _masks():
    p = np.arange(128)[:, None]
    j = np.arange(512)[None, :]
    m = np.zeros((128, 4, 512), dtype=np.float32)
    for d in range(4):
        m[:, d, :] = np.where(d * 128 + p <= j, 0.0, NEG)
    return m


def _core_inputs(core, x, W_dq, W_dkv, W_uq, W_uk, W_uv, W_kr, W_qr, W_o, S):
    b, hg = core // 4, core % 4
    h0 = hg * HPC
    f = np.float32

    def pm(w, nchunk):  # [R, C] -> [128, R//128, C] partition-major
        R, Cc = w.shape
        return np.ascontiguousarray(
            w.reshape(R // 128, 128, Cc).transpose(1, 0, 2)).astype(np.float16)

    heads = np.arange(h0, h0 + HPC)
    rope_x1 = (heads[:, None] * D_ROPE + np.arange(HALF)[None, :]).reshape(-1)
    rope_x2 = rope_x1 + HALF
    nope_cols = (heads[:, None] * D_NOPE + np.arange(D_NOPE)[None, :]).reshape(-1)
    v_cols = (heads[:, None] * D_HEAD + np.arange(D_HEAD)[None, :]).reshape(-1)

    xT = np.ascontiguousarray(x[b].T)                     # [2048, S]
    TCn = S // 512
    cosA, sinA = _rope_tables(S)
    return {
        "xT": np.ascontiguousarray(
            pm(xT, KC).reshape(128, KC, TCn, 512).transpose(2, 0, 1, 3)
        ).reshape(TCn, 128, KC * 512),
        "wdq": pm(W_dq, KC),
        "wdkv": pm(W_dkv, KC),
        "wkrx1": pm(W_kr[:, rope_x1], KC),
        "wkrx2": pm(W_kr[:, rope_x2], KC),
        "wuq": pm(W_uq[:, nope_cols], CC),
        "wuk": pm(W_uk[:, nope_cols], CC),
        "wqrx1": pm(W_qr[:, rope_x1], CC),
        "wqrx2": pm(W_qr[:, rope_x2], CC),
        "wuv": pm(W_uv[:, v_cols], CC),
        "wo": pm(W_o[h0 * D_HEAD:(h0 + HPC) * D_HEAD, :], HPC),
        "cosA": cosA,
        "sinA": sinA,
        "masks": _masks(),
        "onec": np.ones((128, 1), np.float16),
        "oner": np.ones((1, 128), np.float16),
    }


_NC_CACHE = {}


def _get_nc(S):
    if S not in _NC_CACHE:
        _NC_CACHE[S] = build_nc(S)
    return _NC_CACHE[S]


def make_in_maps(inputs, S):
    args = (np.asarray(inputs["x"], np.float32),
            np.asarray(inputs["W_dq"], np.float32),
            np.asarray(inputs["W_dkv"], np.float32),
            np.asarray(inputs["W_uq"], np.float32),
            np.asarray(inputs["W_uk"], np.float32),
            np.asarray(inputs["W_uv"], np.float32),
            np.asarray(inputs["W_kr"], np.float32),
            np.asarray(inputs["W_qr"], np.float32),
            np.asarray(inputs["W_o"], np.float32))
    x, W_dq, W_dkv, W_uq, W_uk, W_uv, W_kr, W_qr, W_o = args
    return [
        _core_inputs(c, x, W_dq, W_dkv, W_uq, W_uk, W_uv, W_kr, W_qr, W_o, S)
        for c in range(8)
    ]


def kernel(x, W_dkv, W_dq, W_uq, W_uk, W_uv, W_kr, W_qr, W_o, _trace=False):
    S = x.shape[1]
    nc = _get_nc(S)
    in_maps = make_in_maps(dict(x=x, W_dq=W_dq, W_dkv=W_dkv, W_uq=W_uq,
                                W_uk=W_uk, W_uv=W_uv, W_kr=W_kr, W_qr=W_qr,
                                W_o=W_o), S)
    res = bass_utils.run_bass_kernel_spmd(nc, in_maps, core_ids=list(range(8)),
                                          trace=_trace)
    out = np.zeros((B, S, H_DIM), np.float32)
    for c in range(8):
        out[c // 4] += res.results[c]["o"]
    if _trace:
        kernel.last_exec_time_ns = res.exec_time_ns
        kernel.last_results = res
    return out



# revision 4
# speedup vs baseline: 1.6508x; 1.6508x over previous
"""MLA (multi-head latent attention) Trainium2 Bass kernel, 8-core SPMD.

Sharding: 2-way data parallel over batch x 4-way tensor parallel over heads.
Core c handles batch b = c // 4 and heads [hg*4, hg*4+4) with hg = c % 4.
Each core computes the full MLA forward for its batch/heads and the partial
out-projection (row-sharded W_o); the host sums the 4 partials per batch.

Layouts on device (partition dim first):
  xT      [128, 16, S]   x[b].T, feature-on-partition
  c_qT    [128, 4, S]    (x @ W_dq).T
  c_kvT   [128, 4, S]
  krx1/2  [128, S]       (x @ W_kr)_cols.T, rope halves split, 4 heads x 32
  qT/kT   [128, 4, S]    per head: rows 0:64 content, 64:128 rope
  v       [128, 16, 512] [token%128, token//128, head*128+d]
  scoresT [128k, 512q]   PSUM; Z accumulated as [1,512] PSUM via ones-matmul
                         per key chunk; 1/Z via reciprocal_approx_fast +
                         gpsimd partition_broadcast; normalize at eviction.
Attention is software-pipelined: scores run 2 chunks ahead of po/pz, the
per-head softmax epilogue is deferred into the next head's chunk loop, and
out-projection jobs drip into later heads so the PE never stalls.
All matmul operands are fp16 (full PE rate, ~1.5e-4 rel err).
"""
import sys

sys.path.insert(0, "/opt/trn_rl_repo")

import numpy as np

import concourse.bacc as bacc
import concourse.mybir as mybir
import concourse.tile as tile
from concourse import bass_utils

H_DIM = 2048
N_HEADS = 16
D_HEAD = 128
D_ROPE = 64
D_NOPE = 64
HALF = D_ROPE // 2          # 32
C_DIM = 512
ROPE_BASE = 10000.0
HPC = 4                     # heads per core
B = 2
S_FULL = 2048
KC = H_DIM // 128           # 16
CC = C_DIM // 128           # 4
SCALE = 1.0 / float(np.sqrt(D_HEAD))
NEG = -1e9

f32r = mybir.dt.float16   # matmul operand dtype (1 cyc/row, FWL)
f32 = mybir.dt.float32
bf16 = mybir.dt.float16


def build_nc(S=S_FULL):
    TC = S // 512            # stage-1 token chunks
    QG = S // 512            # query groups
    TT = S // 128            # token tiles

    nc = bacc.Bacc("TRN2", target_bir_lowering=False, debug=False)

    d_xT = nc.dram_tensor("xT", [TC, 128, KC * 512], f32r, kind="ExternalInput")
    d_wdq = nc.dram_tensor("wdq", [128, KC, C_DIM], f32r, kind="ExternalInput")
    d_wdkv = nc.dram_tensor("wdkv", [128, KC, C_DIM], f32r, kind="ExternalInput")
    d_wkrx1 = nc.dram_tensor("wkrx1", [128, KC, HPC * HALF], f32r, kind="ExternalInput")
    d_wkrx2 = nc.dram_tensor("wkrx2", [128, KC, HPC * HALF], f32r, kind="ExternalInput")
    d_wuq = nc.dram_tensor("wuq", [128, CC, HPC * D_NOPE], f32r, kind="ExternalInput")
    d_wuk = nc.dram_tensor("wuk", [128, CC, HPC * D_NOPE], f32r, kind="ExternalInput")
    d_wqrx1 = nc.dram_tensor("wqrx1", [128, CC, HPC * HALF], f32r, kind="ExternalInput")
    d_wqrx2 = nc.dram_tensor("wqrx2", [128, CC, HPC * HALF], f32r, kind="ExternalInput")
    d_wuv = nc.dram_tensor("wuv", [128, CC, HPC * D_HEAD], f32r, kind="ExternalInput")
    d_wo = nc.dram_tensor("wo", [128, HPC, H_DIM], f32r, kind="ExternalInput")
    d_cos = nc.dram_tensor("cosA", [128, S], bf16, kind="ExternalInput")
    d_sin = nc.dram_tensor("sinA", [128, S], bf16, kind="ExternalInput")
    d_tri = nc.dram_tensor("tri", [128, 128], f32, kind="ExternalInput")
    d_onec = nc.dram_tensor("onec", [128, 1], f32r, kind="ExternalInput")
    d_o = nc.dram_tensor("o", [S, H_DIM], bf16, kind="ExternalOutput")

    with tile.TileContext(nc) as tc:
        _cms = {}

        def popen(name, **kw):
            cm = tc.tile_pool(name=name, **kw)
            _cms[name] = cm
            return cm.__enter__()

        def pclose(*names):
            for n in names:
                _cms.pop(n).__exit__(None, None, None)

        # ---- persistent pools (manual lifetime management) ----
        p_const = popen("const", bufs=1)
        p_ckv = popen("ckv", bufs=1, side="right")
        p_krx = popen("krx", bufs=1, side="right")
        p_cq = popen("cq", bufs=1, side="right")

        onec = p_const.tile([128, 1], f32r, tag="onec")
        nc.sync.dma_start(onec[:], d_onec.ap())

        cq = p_cq.tile([128, CC, S], f32r, tag="cq")
        ckv = p_ckv.tile([128, CC, S], f32r, tag="ckv")
        krx1 = p_krx.tile([128, S], bf16, tag="krx1")
        krx2 = p_krx.tile([128, S], bf16, tag="krx2")

        # ================= stage 1: compressions =================
        p_w1 = popen("w1", bufs=1, side="right")
        p_x = popen("xp", bufs=2, side="right")
        p_ps12 = popen("ps12", bufs=4, space="PSUM")

        wdq = p_w1.tile([128, KC, C_DIM], f32r, tag="wdq")
        nc.sync.dma_start(wdq[:], d_wdq.ap())
        xt0 = p_x.tile([128, KC, 512], f32r, tag="xt")
        nc.sync.dma_start(xt0[:], d_xT.ap()[0])
        wdkv = p_w1.tile([128, KC, C_DIM], f32r, tag="wdkv")
        nc.sync.dma_start(wdkv[:], d_wdkv.ap())
        wkrx1 = p_w1.tile([128, KC, HPC * HALF], f32r, tag="wkrx1")
        nc.sync.dma_start(wkrx1[:], d_wkrx1.ap())
        wkrx2 = p_w1.tile([128, KC, HPC * HALF], f32r, tag="wkrx2")
        nc.sync.dma_start(wkrx2[:], d_wkrx2.ap())

        for tci in range(TC):
            ts = slice(tci * 512, (tci + 1) * 512)
            if tci == 0:
                xt = xt0
            else:
                xt = p_x.tile([128, KC, 512], f32r, tag="xt")
                nc.sync.dma_start(xt[:], d_xT.ap()[tci])
            for w_sb, out_sb in ((wdq, cq), (wdkv, ckv)):
                for m in range(CC):
                    ps = p_ps12.tile([128, 512], f32, tag="ps")
                    for k in range(KC):
                        nc.tensor.matmul(
                            ps[:],
                            w_sb[:, k, m * 128:(m + 1) * 128],
                            xt[:, k, :],
                            start=(k == 0), stop=(k == KC - 1),
                        )
                    nc.scalar.copy(out_sb[:, m, ts], ps[:])
            for w_sb, out_sb in ((wkrx1, krx1), (wkrx2, krx2)):
                ps = p_ps12.tile([128, 512], f32, tag="ps")
                for k in range(KC):
                    nc.tensor.matmul(
                        ps[:], w_sb[:, k, :], xt[:, k, :],
                        start=(k == 0), stop=(k == KC - 1),
                    )
                nc.scalar.copy(out_sb[:, ts], ps[:])

        pclose("xp", "w1")

        # ================= stage 2: up-projections + rope =================
        p_w2 = popen("w2", bufs=1, side="right")
        p_cs = popen("cs", bufs=1, side="right")
        p_tmp = popen("tmp", bufs=1, side="right")
        p_qT = popen("qT", bufs=1)

        wuq = p_w2.tile([128, CC, HPC * D_NOPE], f32r, tag="wuq")
        nc.sync.dma_start(wuq[:], d_wuq.ap())
        wuk = p_w2.tile([128, CC, HPC * D_NOPE], f32r, tag="wuk")
        nc.sync.dma_start(wuk[:], d_wuk.ap())
        wqrx1 = p_w2.tile([128, CC, HPC * HALF], f32r, tag="wqrx1")
        nc.sync.dma_start(wqrx1[:], d_wqrx1.ap())
        wqrx2 = p_w2.tile([128, CC, HPC * HALF], f32r, tag="wqrx2")
        nc.sync.dma_start(wqrx2[:], d_wqrx2.ap())
        wuv = p_w2.tile([128, CC, HPC * D_HEAD], f32r, tag="wuv")
        nc.sync.dma_start(wuv[:], d_wuv.ap())
        cosA = p_cs.tile([128, S], bf16, tag="cosA")
        nc.sync.dma_start(cosA[:], d_cos.ap())
        sinA = p_cs.tile([128, S], bf16, tag="sinA")
        nc.sync.dma_start(sinA[:], d_sin.ap())

        qT = p_qT.tile([128, HPC, S], f32r, tag="qT")

        def up_nope(w_sb, src, dst, g):
            gs = slice(g * 512, (g + 1) * 512)
            for m2 in range(2):
                ps = p_ps12.tile([128, 512], f32, tag="ps")
                for k in range(CC):
                    nc.tensor.matmul(
                        ps[:], w_sb[:, k, m2 * 128:(m2 + 1) * 128],
                        src[:, k, gs], start=(k == 0), stop=(k == CC - 1),
                    )
                nc.scalar.copy(dst[0:64, 2 * m2, gs], ps[0:64, :])
                nc.scalar.copy(dst[0:64, 2 * m2 + 1, gs], ps[64:128, :])

        def rope_from_psum(x1ap, x2ap, dst, g):
            # x1/x2: [128 = 4h*32, 512] (psum or sbuf); writes rope rows of dst
            gs = slice(g * 512, (g + 1) * 512)
            t1 = p_tmp.tile([128, 512], f32, tag="t1")
            t2 = p_tmp.tile([128, 512], f32, tag="t2")
            o1 = p_tmp.tile([128, 512], f32r, tag="o1")
            o2 = p_tmp.tile([128, 512], f32r, tag="o2")
            nc.vector.tensor_mul(t1[:], x1ap, cosA[:, gs])
            nc.vector.tensor_mul(t2[:], x2ap, sinA[:, gs])
            nc.vector.tensor_sub(o1[:], t1[:], t2[:])
            nc.vector.tensor_mul(t1[:], x1ap, sinA[:, gs])
            nc.vector.tensor_mul(t2[:], x2ap, cosA[:, gs])
            nc.vector.tensor_add(o2[:], t1[:], t2[:])
            for h in range(HPC):
                hs = slice(h * HALF, (h + 1) * HALF)
                nc.scalar.copy(dst[64:96, h, gs], o1[hs, :])
                nc.scalar.copy(dst[96:128, h, gs], o2[hs, :])

        # ---- queries (need cq) ----
        for g in range(QG):
            gs = slice(g * 512, (g + 1) * 512)
            up_nope(wuq, cq, qT, g)
            ps1 = p_ps12.tile([128, 512], f32, tag="ps")
            for k in range(CC):
                nc.tensor.matmul(ps1[:], wqrx1[:, k, :], cq[:, k, gs],
                                 start=(k == 0), stop=(k == CC - 1))
            ps2 = p_ps12.tile([128, 512], f32, tag="ps")
            for k in range(CC):
                nc.tensor.matmul(ps2[:], wqrx2[:, k, :], cq[:, k, gs],
                                 start=(k == 0), stop=(k == CC - 1))
            rope_from_psum(ps1[:], ps2[:], qT, g)

        # ---- keys + values (need ckv, krx) ----
        p_kT = popen("kT", bufs=1)
        p_v = popen("vp", bufs=1)
        kT = p_kT.tile([128, HPC, S], f32r, tag="kT")
        v_sb = p_v.tile([128, TT, 512], f32r, tag="v")

        for g in range(QG):
            gs = slice(g * 512, (g + 1) * 512)
            up_nope(wuk, ckv, kT, g)
            rope_from_psum(krx1[:, gs], krx2[:, gs], kT, g)
        for tt in range(TT):
            ps = p_ps12.tile([128, 512], f32, tag="ps")
            for k in range(CC):
                nc.tensor.matmul(ps[:], ckv[:, k, tt * 128:(tt + 1) * 128],
                                 wuv[:, k, :], start=(k == 0), stop=(k == CC - 1))
            nc.scalar.copy(v_sb[:, tt, :], ps[:])

        pclose("tmp", "cs", "w2", "cq", "krx", "ckv", "ps12")

        # ================= stage 3+4: attention + out-projection =================
        # PSUM banks: sc(4) + po(2) + pz(2) = 8
        p_msk = popen("msk", bufs=1)
        p_wo = popen("wo", bufs=1)
        p_et = popen("et", bufs=6)
        p_att = popen("att", bufs=2)
        p_rz = popen("rz", bufs=2)
        p_rb = popen("rb", bufs=2)
        p_o = popen("op", bufs=3)
        p_sc = popen("sc", bufs=4, space="PSUM")
        p_po = popen("po", bufs=2, space="PSUM")
        p_pz = popen("pz", bufs=2, space="PSUM")

        tri = p_msk.tile([128, 128], f32, tag="tri")
        nc.sync.dma_start(tri[:], d_tri.ap())
        wo = p_wo.tile([128, HPC, H_DIM], f32r, tag="wo")
        nc.sync.dma_start(wo[:], d_wo.ap())

        # deferred work queues (emission-order software pipelining)
        wo_jobs = []
        pend = []

        def emit_wo_job(attn_t, g, t4, nck):
            def run():
                pso = p_sc.tile([128, 512], f32, tag="ps")
                for h in range(HPC):
                    nc.tensor.matmul(
                        pso[:], attn_t[:, h, t4 * 128:(t4 + 1) * 128],
                        wo[:, h, nck * 512:(nck + 1) * 512],
                        start=(h == 0), stop=(h == HPC - 1),
                    )
                ot = p_o.tile([128, 512], bf16, tag="ot")
                nc.vector.tensor_copy(ot[:], pso[:])
                tt = g * 4 + t4
                nc.sync.dma_start(
                    d_o.ap()[tt * 128:(tt + 1) * 128, nck * 512:(nck + 1) * 512],
                    ot[:])
            return run

        def emit_epilogue(job):
            po, pz, attn_t, h = job
            rz = p_rz.tile([1, 512], f32, tag="rz")
            nc.vector.reciprocal_approx_fast(rz[:], pz[:])
            rb = p_rb.tile([128, 512], f32, tag="rb")
            nc.gpsimd.partition_broadcast(rb[:], rz[:], channels=128)
            nc.vector.tensor_mul(attn_t[:, h, :], po[:], rb[:])

        for g in range(QG):
            attn_g = p_att.tile([128, HPC, 512], f32r, tag="attn")
            for h in range(HPC):
                nkt = 4 * (g + 1)
                po = p_po.tile([128, 512], f32, tag="po")
                pz = p_pz.tile([1, 512], f32, tag="pz")

                def emit_pz_po(item):
                    et, lo, kt = item
                    nc.tensor.matmul(po[:, lo:], v_sb[:, kt, h * 128:(h + 1) * 128],
                                     et[:, lo:], start=(kt == 0), stop=(kt == nkt - 1),
                                     skip_group_check=True)
                    nc.tensor.matmul(pz[:, lo:], onec[:], et[:, lo:],
                                     start=(kt == 0), stop=(kt == nkt - 1),
                                     skip_group_check=True)

                hist = []
                for kt in range(nkt):
                    d = kt - 4 * g
                    lo = 128 * d if d >= 1 else 0
                    ps = p_sc.tile([128, 512], f32, tag="ps")
                    nc.tensor.matmul(ps[:, lo:], kT[:, h, kt * 128:(kt + 1) * 128],
                                     qT[:, h, g * 512 + lo:(g + 1) * 512],
                                     start=True, stop=True)
                    if d >= 0:
                        nc.vector.tensor_add(ps[:, 128 * d:128 * d + 128],
                                             ps[:, 128 * d:128 * d + 128], tri[:])
                    et = p_et.tile([128, 512], f32r, tag="et")
                    nc.scalar.activation(et[:, lo:], ps[:, lo:],
                                         mybir.ActivationFunctionType.Exp,
                                         scale=SCALE)
                    hist.append((et, lo, kt))
                    if len(hist) > 2:
                        emit_pz_po(hist.pop(0))
                    # drip-feed deferred epilogue / out-proj into PE bubbles
                    if kt == 1 and pend:
                        emit_epilogue(pend.pop(0))
                    elif kt >= 2 and kt % 2 == 0 and wo_jobs:
                        wo_jobs.pop(0)()
                for item in hist:
                    emit_pz_po(item)
                pend.append((po, pz, attn_g, h))
            for t4 in range(4):
                for nck in range(4):
                    wo_jobs.append(emit_wo_job(attn_g, g, t4, nck))

        while pend:
            emit_epilogue(pend.pop(0))
        while wo_jobs:
            wo_jobs.pop(0)()

        pclose("pz", "po", "sc", "op", "rb", "rz", "att", "et",
               "wo", "msk", "vp", "kT", "qT", "const")

    nc.compile()
    return nc


# ================= host-side prep =================

def _rope_tables(S):
    inv_freq = 1.0 / (ROPE_BASE ** (np.arange(HALF, dtype=np.float64) / HALF))
    ang = np.arange(S, dtype=np.float64)[:, None] * inv_freq[None, :]   # [S, 32]
    cosA = np.tile(np.cos(ang).T, (4, 1)).astype(np.float16)           # [128, S]
    sinA = np.tile(np.sin(ang).T, (4, 1)).astype(np.float16)
    return cosA, sinA


def _tri():
    p = np.arange(128)[:, None]
    j = np.arange(128)[None, :]
    return np.where(p <= j, 0.0, NEG).astype(np.float32)


def _core_inputs(core, x, W_dq, W_dkv, W_uq, W_uk, W_uv, W_kr, W_qr, W_o, S):
    b, hg = core // 4, core % 4
    h0 = hg * HPC

    def pm(w, nchunk):  # [R, C] -> [128, R//128, C] partition-major
        R, Cc = w.shape
        return np.ascontiguousarray(
            w.reshape(R // 128, 128, Cc).transpose(1, 0, 2)).astype(np.float16)

    heads = np.arange(h0, h0 + HPC)
    rope_x1 = (heads[:, None] * D_ROPE + np.arange(HALF)[None, :]).reshape(-1)
    rope_x2 = rope_x1 + HALF
    nope_cols = (heads[:, None] * D_NOPE + np.arange(D_NOPE)[None, :]).reshape(-1)
    v_cols = (heads[:, None] * D_HEAD + np.arange(D_HEAD)[None, :]).reshape(-1)

    xT = np.ascontiguousarray(x[b].T)                     # [2048, S]
    TCn = S // 512
    cosA, sinA = _rope_tables(S)
    return {
        "xT": np.ascontiguousarray(
            pm(xT, KC).reshape(128, KC, TCn, 512).transpose(2, 0, 1, 3)
        ).reshape(TCn, 128, KC * 512),
        "wdq": pm(W_dq, KC),
        "wdkv": pm(W_dkv, KC),
        "wkrx1": pm(W_kr[:, rope_x1], KC),
        "wkrx2": pm(W_kr[:, rope_x2], KC),
        "wuq": pm(W_uq[:, nope_cols], CC),
        "wuk": pm(W_uk[:, nope_cols], CC),
        "wqrx1": pm(W_qr[:, rope_x1], CC),
        "wqrx2": pm(W_qr[:, rope_x2], CC),
        "wuv": pm(W_uv[:, v_cols], CC),
        "wo": pm(W_o[h0 * D_HEAD:(h0 + HPC) * D_HEAD, :], HPC),
        "cosA": cosA,
        "sinA": sinA,
        "tri": _tri(),
        "onec": np.ones((128, 1), np.float16),
    }


_NC_CACHE = {}


def _get_nc(S):
    if S not in _NC_CACHE:
        _NC_CACHE[S] = build_nc(S)
    return _NC_CACHE[S]


def make_in_maps(inputs, S):
    args = (np.asarray(inputs["x"], np.float32),
            np.asarray(inputs["W_dq"], np.float32),
            np.asarray(inputs["W_dkv"], np.float32),
            np.asarray(inputs["W_uq"], np.float32),
            np.asarray(inputs["W_uk"], np.float32),
            np.asarray(inputs["W_uv"], np.float32),
            np.asarray(inputs["W_kr"], np.float32),
            np.asarray(inputs["W_qr"], np.float32),
            np.asarray(inputs["W_o"], np.float32))
    x, W_dq, W_dkv, W_uq, W_uk, W_uv, W_kr, W_qr, W_o = args
    return [
        _core_inputs(c, x, W_dq, W_dkv, W_uq, W_uk, W_uv, W_kr, W_qr, W_o, S)
        for c in range(8)
    ]


def kernel(x, W_dkv, W_dq, W_uq, W_uk, W_uv, W_kr, W_qr, W_o, _trace=False):
    S = x.shape[1]
    nc = _get_nc(S)
    in_maps = make_in_maps(dict(x=x, W_dq=W_dq, W_dkv=W_dkv, W_uq=W_uq,
                                W_uk=W_uk, W_uv=W_uv, W_kr=W_kr, W_qr=W_qr,
                                W_o=W_o), S)
    res = bass_utils.run_bass_kernel_spmd(nc, in_maps, core_ids=list(range(8)),
                                          trace=_trace)
    out = np.zeros((B, S, H_DIM), np.float32)
    for c in range(8):
        out[c // 4] += np.asarray(res.results[c]["o"], np.float32)
    if _trace:
        kernel.last_exec_time_ns = res.exec_time_ns
        kernel.last_results = res
    return out
